# revision 19
# baseline (speedup 1.0000x reference)
"""BlockAttentionResidual Trainium2 kernel (plane-major fp16 pipeline).

Math (per (b,t) row, V slice v_n of length D, n = 0..7):
    ssq_n = sum(v_n^2)
    rms_n = rsqrt(ssq_n / D + eps)
    logit_n = rms_n * dot(v_n, qw)        with qw = key_norm_weight * pseudo_query
    w = softmax(logit)                     over n
    out = sum_n w_n * v_n

Sharding: rows (B*T flattened) split evenly across 8 cores; (D,) params
replicated. No cross-core communication.

Per-core layout: blocks of 256 rows. For each block, 8 plane tiles
[128, 2D] with partition p holding HBM-contiguous rows (2p, 2p+1) of one
plane -> every load is a fully contiguous 2 MiB HBM read. Loads go
through SWDGE (gpsimd) with an inline f32 -> fp16 cast: HBM reads stay
f32 (required bytes) but every downstream engine pass runs at 16-bit
rates (DVE 2x mode, ScalarE 2x, half-size LDWEIGHTS) and SBUF tiles
halve, buying a 2.5-block prefetch depth.
  - ssq: ScalarE activation(Square, accum_out) per row-half
  - dot: VectorE scalar_tensor_tensor(mult, accum_out) per row-half
  - rms = exp(-0.5*ln(ssq/D+eps)) on ScalarE
  - softmax over n: plane index is on the free axis ([128, 8] tiles),
    direct vector ops, no transposes
  - weighted sum: PE matmul, fp16 diag stationaries diag(w_eo[:, n])
    built by per-plane tensor_scalar; 8 accumulating matmuls per
    512-chunk per (parity, D-half)
  - output staged bf16 in SBUF (halves store traffic), host upcasts
Precision (numpy-simulated): fp16 x/qw dot noise sigma~0.03 on logits,
fp16 weights, bf16 store -> rel err ~2.3e-3 (gate 2e-2).
DMA rings: loads on SWDGE, consts + stores on the scalar HWDGE ring.
"""

import os
import sys

for _p in ("/opt/trn_rl_repo",):
    if _p not in sys.path and os.path.isdir(_p):
        sys.path.append(_p)

import numpy as np

import concourse.bass as bass
import concourse.tile as tile
from concourse import bacc, mybir
from concourse.bass_utils import run_bass_kernel_spmd

N_CORES = 8
N = 8          # depth entries (softmax axis)
B = 4
T = 2048
D = 2048
R_TOTAL = B * T            # 8192 rows
RPC = R_TOTAL // N_CORES   # 1024 rows per core
BR = 256                   # rows per block (2 rows per partition)
EPS = 1e-6
NCHUNK = 512               # matmul moving free-dim chunk

F32 = mybir.dt.float32
BF16 = mybir.dt.bfloat16
F16 = mybir.dt.float16
ALU = mybir.AluOpType
ACTF = mybir.ActivationFunctionType


def build_program(rows_per_core=RPC, debug=False, xbufs=19):
    """Build the per-core Bass program (identical on all cores)."""
    nb = rows_per_core // BR           # blocks per core
    nc = bacc.Bacc(
        "TRN2", target_bir_lowering=False, debug=debug, num_devices=N_CORES
    )

    v_dram = nc.dram_tensor("V", (N, rows_per_core, D), F32, kind="ExternalInput").ap()
    qw_dram = nc.dram_tensor("QW", (128, D), F16, kind="ExternalInput").ap()
    id_dram = nc.dram_tensor("ID", (128, 128), F16, kind="ExternalInput").ap()
    out_dram = nc.dram_tensor(
        "OUT", (rows_per_core, D), BF16, kind="ExternalOutput"
    ).ap()

    with tile.TileContext(nc) as tc:
        with (
            tc.tile_pool(name="consts", bufs=1) as consts,
            tc.tile_pool(name="xpool", bufs=xbufs) as xpool,
            tc.tile_pool(name="scratch", bufs=1) as scratch,
            tc.tile_pool(name="outpool", bufs=1) as outpool,
            tc.tile_pool(name="smalls", bufs=3) as smalls,
            tc.tile_pool(name="wdpool", bufs=2) as wdpool,
            tc.tile_pool(name="psum_big", bufs=2, space="PSUM") as psum_big_pool,
        ):
            qw_sb = consts.tile([128, D], F16)
            nc.scalar.dma_start(qw_sb[:], qw_dram[:])
            id_sb = consts.tile([128, 128], F16)
            nc.scalar.dma_start(id_sb[:], id_dram[:])
            zero_sb = consts.tile([128, 1], F32)
            nc.vector.memset(zero_sb[:], 0.0)
            eps_sb = consts.tile([128, 1], F32)
            nc.vector.memset(eps_sb[:], EPS)

            def emit_block(b):
                xt = []
                dots2 = smalls.tile([128, 2 * N], F32, tag="dots2")
                ssqs2 = smalls.tile([128, 2 * N], F32, tag="ssqs2")
                for n in range(N):
                    x = xpool.tile([128, 2 * D], F16, tag="x")
                    # partition p <- rows (BR*b + 2p, BR*b + 2p + 1) of plane n
                    src = (
                        v_dram[n, BR * b : BR * (b + 1), :]
                        .rearrange("(p two) d -> p (two d)", two=2)
                    )
                    nc.gpsimd.dma_start(x[:], src)
                    xt.append(x)

                    for eo in range(2):
                        xh = x[:, D * eo : D * (eo + 1)]
                        prod = scratch.tile([128, D], F16, tag="prod")
                        nc.vector.scalar_tensor_tensor(
                            out=prod[:],
                            in0=xh,
                            scalar=1.0,
                            in1=qw_sb[:],
                            op0=ALU.mult,
                            op1=ALU.mult,
                            accum_out=dots2[:, N * eo + n : N * eo + n + 1],
                        )
                        sq = scratch.tile([128, D], F16, tag="sq")
                        nc.scalar.activation(
                            sq[:], xh, ACTF.Square, bias=zero_sb[:],
                            accum_out=ssqs2[:, N * eo + n : N * eo + n + 1],
                        )

                # softmax over n, both parities merged into [128, 2*N] ops
                # (cols = 8*eo + n); rms via sqrt + reciprocal
                sqs = smalls.tile([128, 2 * N], F32, tag="sqs")
                nc.scalar.activation(
                    sqs[:], ssqs2[:], ACTF.Sqrt, bias=eps_sb[:], scale=1.0 / D
                )
                rms = smalls.tile([128, 2 * N], F32, tag="rms")
                nc.vector.reciprocal(rms[:], sqs[:])
                logits = smalls.tile([128, 2 * N], F32, tag="logits")
                nc.vector.tensor_mul(logits[:], dots2[:], rms[:])
                l3 = logits[:].rearrange("p (e n) -> p e n", n=N)
                negmax = smalls.tile([128, 2], F32, tag="negmax")
                nc.vector.tensor_reduce(
                    negmax[:], l3, axis=mybir.AxisListType.X,
                    op=ALU.max, negate=True,
                )
                shifted = smalls.tile([128, 2 * N], F32, tag="shifted")
                s3 = shifted[:].rearrange("p (e n) -> p e n", n=N)
                nc.vector.tensor_tensor(
                    s3, l3, negmax[:].unsqueeze(2).broadcast_to([128, 2, N]),
                    ALU.add,
                )
                expd = smalls.tile([128, 2 * N], F32, tag="expd")
                nc.scalar.activation(expd[:], shifted[:], ACTF.Exp, bias=zero_sb[:])
                e3 = expd[:].rearrange("p (e n) -> p e n", n=N)
                sums = smalls.tile([128, 2], F32, tag="sums")
                nc.vector.tensor_reduce(
                    sums[:], e3, axis=mybir.AxisListType.X, op=ALU.add
                )
                rsums = smalls.tile([128, 2], F32, tag="rsums")
                nc.vector.reciprocal(rsums[:], sums[:])
                wts = smalls.tile([128, 2 * N], F32, tag="wts")
                w3 = wts[:].rearrange("p (e n) -> p e n", n=N)
                nc.vector.tensor_tensor(
                    w3, e3, rsums[:].unsqueeze(2).broadcast_to([128, 2, N]),
                    ALU.mult,
                )
                # wd[:, 128n+m] = wts[:, 8eo+n] * id[:, m] -> diag
                # stationaries, built on the (otherwise idle) gpsimd engine
                wds = []
                for eo in range(2):
                    wd = wdpool.tile([128, N * 128], F16, tag=f"wd{eo}")
                    wd3 = wd[:].rearrange("p (n m) -> p n m", m=128)
                    nc.gpsimd.tensor_tensor(
                        wd3,
                        wts[:, N * eo : N * (eo + 1)]
                        .unsqueeze(2).broadcast_to([128, N, 128]),
                        id_sb[:].unsqueeze(1).broadcast_to([128, N, 128]),
                        ALU.mult,
                    )
                    wds.append(wd)

                # weighted sum: per (parity, D-half), 8 accumulating diag
                # matmuls per 512-chunk
                osb = outpool.tile([128, 2 * D], BF16, tag="osb")
                for eo in range(2):
                    for h in range(2):
                        psb = psum_big_pool.tile([128, D // 2], F32, tag="psb")
                        for n in range(N):
                            lhsT = wds[eo][:, 128 * n : 128 * (n + 1)]
                            for kk in range(D // NCHUNK // 2):
                                k = h * (D // NCHUNK // 2) + kk
                                nc.tensor.matmul(
                                    psb[:, NCHUNK * kk : NCHUNK * (kk + 1)],
                                    lhsT,
                                    xt[n][:, D * eo + NCHUNK * k
                                           : D * eo + NCHUNK * (k + 1)],
                                    start=(n == 0),
                                    stop=(n == N - 1),
                                )
                        # split the PSUM->SBUF copies across ACT and DVE
                        eng = nc.scalar.copy if h == 0 else nc.vector.tensor_copy
                        eng(
                            osb[:, D * eo + h * (D // 2)
                                   : D * eo + (h + 1) * (D // 2)],
                            psb[:],
                        )
                # partition p holds rows (2p, 2p+1): contiguous 1 MiB store
                dst = (
                    out_dram[BR * b : BR * (b + 1), :]
                    .rearrange("(p two) d -> p (two d)", two=2)
                )
                nc.scalar.dma_start(dst, osb[:])

            def emit_half_block(r0):
                """128-row block, one row per partition ([128, D] tiles).

                Used for the final rows so the end-of-kernel dependency
                chain (softmax -> matmuls -> store) is half as long.
                """
                xt = []
                dots = smalls.tile([128, N], F32, tag="dotsh")
                ssqs = smalls.tile([128, N], F32, tag="ssqsh")
                for n in range(N):
                    xfull = xpool.tile([128, 2 * D], F16, tag="x")
                    x = xfull[:, 0:D]
                    nc.gpsimd.dma_start(x, v_dram[n, r0 : r0 + 128, :])
                    xt.append(xfull)
                    prod = scratch.tile([128, D], F16, tag="prod")
                    nc.vector.scalar_tensor_tensor(
                        out=prod[:], in0=x, scalar=1.0, in1=qw_sb[:],
                        op0=ALU.mult, op1=ALU.mult,
                        accum_out=dots[:, n : n + 1],
                    )
                    sq = scratch.tile([128, D], F16, tag="sq")
                    nc.scalar.activation(
                        sq[:], x, ACTF.Square, bias=zero_sb[:],
                        accum_out=ssqs[:, n : n + 1],
                    )

                lns = smalls.tile([128, N], F32, tag="lnsh")
                nc.scalar.activation(
                    lns[:], ssqs[:], ACTF.Ln, bias=eps_sb[:], scale=1.0 / D
                )
                rms = smalls.tile([128, N], F32, tag="rmsh")
                nc.scalar.activation(
                    rms[:], lns[:], ACTF.Exp, bias=zero_sb[:], scale=-0.5
                )
                logits = smalls.tile([128, N], F32, tag="logitsh")
                nc.vector.tensor_mul(logits[:], dots[:], rms[:])
                negmax = smalls.tile([128, 1], F32, tag="negmaxh")
                nc.vector.tensor_reduce(
                    negmax[:], logits[:], axis=mybir.AxisListType.X,
                    op=ALU.max, negate=True,
                )
                shifted = smalls.tile([128, N], F32, tag="shiftedh")
                nc.vector.tensor_tensor(
                    shifted[:], logits[:], negmax[:].broadcast_to([128, N]),
                    ALU.add,
                )
                expd = smalls.tile([128, N], F32, tag="expdh")
                nc.scalar.activation(expd[:], shifted[:], ACTF.Exp, bias=zero_sb[:])
                sums = smalls.tile([128, 1], F32, tag="sumsh")
                nc.vector.tensor_reduce(
                    sums[:], expd[:], axis=mybir.AxisListType.X, op=ALU.add
                )
                rsums = smalls.tile([128, 1], F32, tag="rsumsh")
                nc.vector.reciprocal(rsums[:], sums[:])
                wts = smalls.tile([128, N], F32, tag="wtsh")
                nc.vector.tensor_tensor(
                    wts[:], expd[:], rsums[:].broadcast_to([128, N]), ALU.mult
                )
                wd = wdpool.tile([128, N * 128], F16, tag="wdh")
                for n in range(N):
                    nc.vector.tensor_scalar(
                        out=wd[:, 128 * n : 128 * (n + 1)],
                        in0=id_sb[:],
                        scalar1=wts[:, n : n + 1],
                        scalar2=None,
                        op0=ALU.mult,
                    )

                osb = outpool.tile([128, D], BF16, tag="osbh")
                for h in range(2):
                    psb = psum_big_pool.tile([128, D // 2], F32, tag="psb")
                    for n in range(N):
                        lhsT = wd[:, 128 * n : 128 * (n + 1)]
                        for kk in range(D // NCHUNK // 2):
                            k = h * (D // NCHUNK // 2) + kk
                            nc.tensor.matmul(
                                psb[:, NCHUNK * kk : NCHUNK * (kk + 1)],
                                lhsT,
                                xt[n][:, NCHUNK * k : NCHUNK * (k + 1)],
                                start=(n == 0),
                                stop=(n == N - 1),
                            )
                    eng = nc.scalar.copy if h == 0 else nc.vector.tensor_copy
                    eng(osb[:, h * (D // 2) : (h + 1) * (D // 2)], psb[:])
                nc.scalar.dma_start(out_dram[r0 : r0 + 128, :], osb[:])

            for b in range(nb - 1):
                emit_block(b)
            emit_half_block(BR * (nb - 1))
            emit_half_block(BR * (nb - 1) + 128)

    nc.compile()
    return nc


def prepare_in_maps(V, key_norm_weight, pseudo_query, rows_per_core=RPC,
                    n_cores=N_CORES):
    qw = (np.asarray(key_norm_weight, dtype=np.float32)
          * np.asarray(pseudo_query, dtype=np.float32)).astype(np.float16)
    qw_b = np.ascontiguousarray(np.broadcast_to(qw, (128, D)))
    ident = np.eye(128, dtype=np.float16)
    vf = np.ascontiguousarray(np.asarray(V, dtype=np.float32)).reshape(N, -1, D)
    in_maps = []
    for c in range(n_cores):
        sl = np.ascontiguousarray(
            vf[:, c * rows_per_core : (c + 1) * rows_per_core, :]
        )
        in_maps.append({"V": sl, "QW": qw_b, "ID": ident})
    return in_maps


_PROGRAM_CACHE = {}


def _get_program():
    key = (RPC,)
    if key not in _PROGRAM_CACHE:
        _PROGRAM_CACHE[key] = build_program(RPC, debug=False)
    return _PROGRAM_CACHE[key]


def run(V, key_norm_weight, pseudo_query, trace=False, **trace_kwargs):
    nc = _get_program()
    in_maps = prepare_in_maps(V, key_norm_weight, pseudo_query)
    res = run_bass_kernel_spmd(
        nc, in_maps, list(range(N_CORES)), trace=trace, **trace_kwargs
    )
    out = np.empty((R_TOTAL, D), dtype=np.float32)
    for c in range(N_CORES):
        out[c * RPC : (c + 1) * RPC, :] = np.asarray(
            res.results[c]["OUT"]
        ).astype(np.float32)
    return out.reshape(B, T, D), res


def kernel(V, key_norm_weight, pseudo_query):
    out, _ = run(V, key_norm_weight, pseudo_query, trace=False)
    return out


# revision 20
# speedup vs baseline: 1.0429x; 1.0429x over previous
"""BlockAttentionResidual Trainium2 kernel (plane-major fp16 pipeline).

Math (per (b,t) row, V slice v_n of length D, n = 0..7):
    ssq_n = sum(v_n^2)
    rms_n = rsqrt(ssq_n / D + eps)
    logit_n = rms_n * dot(v_n, qw)        with qw = key_norm_weight * pseudo_query
    w = softmax(logit)                     over n
    out = sum_n w_n * v_n

Sharding: rows (B*T flattened) split evenly across 8 cores; (D,) params
replicated. No cross-core communication.

Per-core layout: blocks of 256 rows. For each block, 8 plane tiles
[128, 2D] with partition p holding HBM-contiguous rows (2p, 2p+1) of one
plane -> every load is a fully contiguous 2 MiB HBM read. Loads go
through SWDGE (gpsimd) with an inline f32 -> fp16 cast: HBM reads stay
f32 (required bytes) but every downstream engine pass runs at 16-bit
rates (DVE 2x mode, ScalarE 2x, half-size LDWEIGHTS) and SBUF tiles
halve, buying a 2.5-block prefetch depth.
  - ssq: ScalarE activation(Square, accum_out) per row-half
  - dot: VectorE scalar_tensor_tensor(mult, accum_out) per row-half
  - rms = exp(-0.5*ln(ssq/D+eps)) on ScalarE
  - softmax over n: plane index is on the free axis ([128, 8] tiles),
    direct vector ops, no transposes
  - weighted sum: PE matmul, fp16 diag stationaries diag(w_eo[:, n])
    built by per-plane tensor_scalar; 8 accumulating matmuls per
    512-chunk per (parity, D-half)
  - output staged bf16 in SBUF (halves store traffic), host upcasts
Precision (numpy-simulated): fp16 x/qw dot noise sigma~0.03 on logits,
fp16 weights, bf16 store -> rel err ~2.3e-3 (gate 2e-2).
DMA rings: loads on SWDGE, consts + stores on the scalar HWDGE ring.
"""

import os
import sys

for _p in ("/opt/trn_rl_repo",):
    if _p not in sys.path and os.path.isdir(_p):
        sys.path.append(_p)

import numpy as np

import concourse.bass as bass
import concourse.tile as tile
from concourse import bacc, mybir
from concourse.bass_utils import run_bass_kernel_spmd

N_CORES = 8
N = 8          # depth entries (softmax axis)
B = 4
T = 2048
D = 2048
R_TOTAL = B * T            # 8192 rows
RPC = R_TOTAL // N_CORES   # 1024 rows per core
BR = 256                   # rows per block (2 rows per partition)
EPS = 1e-6
NCHUNK = 512               # matmul moving free-dim chunk

F32 = mybir.dt.float32
BF16 = mybir.dt.bfloat16
F16 = mybir.dt.float16
ALU = mybir.AluOpType
ACTF = mybir.ActivationFunctionType


def build_program(rows_per_core=RPC, debug=False, xbufs=19):
    """Build the per-core Bass program (identical on all cores)."""
    nb = rows_per_core // BR           # blocks per core
    nc = bacc.Bacc(
        "TRN2", target_bir_lowering=False, debug=debug, num_devices=N_CORES
    )

    v_dram = nc.dram_tensor("V", (N, rows_per_core, D), F32, kind="ExternalInput").ap()
    qw_dram = nc.dram_tensor("QW", (128, D), F16, kind="ExternalInput").ap()
    id_dram = nc.dram_tensor("ID", (128, 128), F16, kind="ExternalInput").ap()
    out_dram = nc.dram_tensor(
        "OUT", (rows_per_core, D), BF16, kind="ExternalOutput"
    ).ap()

    with tile.TileContext(nc) as tc:
        with (
            tc.tile_pool(name="consts", bufs=1) as consts,
            tc.tile_pool(name="xpool", bufs=xbufs) as xpool,
            tc.tile_pool(name="scratch", bufs=1) as scratch,
            tc.tile_pool(name="outpool", bufs=1) as outpool,
            tc.tile_pool(name="smalls", bufs=3) as smalls,
            tc.tile_pool(name="wdpool", bufs=2) as wdpool,
            tc.tile_pool(name="psum_big", bufs=2, space="PSUM") as psum_big_pool,
        ):
            qw_sb = consts.tile([128, D], F16)
            nc.scalar.dma_start(qw_sb[:], qw_dram[:])
            id_sb = consts.tile([128, 128], F16)
            nc.scalar.dma_start(id_sb[:], id_dram[:])
            zero_sb = consts.tile([128, 1], F32)
            nc.vector.memset(zero_sb[:], 0.0)
            eps_sb = consts.tile([128, 1], F32)
            nc.vector.memset(eps_sb[:], EPS)

            def emit_block(b):
                xt = []
                dots2 = smalls.tile([128, 2 * N], F32, tag="dots2")
                ssqs2 = smalls.tile([128, 2 * N], F32, tag="ssqs2")
                for n in range(N):
                    x = xpool.tile([128, 2 * D], F16, tag="x")
                    # partition p <- rows (BR*b + 2p, BR*b + 2p + 1) of plane n
                    src = (
                        v_dram[n, BR * b : BR * (b + 1), :]
                        .rearrange("(p two) d -> p (two d)", two=2)
                    )
                    nc.gpsimd.dma_start(x[:], src)
                    xt.append(x)

                    for eo in range(2):
                        xh = x[:, D * eo : D * (eo + 1)]
                        prod = scratch.tile([128, D], F16, tag="prod")
                        nc.vector.scalar_tensor_tensor(
                            out=prod[:],
                            in0=xh,
                            scalar=1.0,
                            in1=qw_sb[:],
                            op0=ALU.mult,
                            op1=ALU.mult,
                            accum_out=dots2[:, N * eo + n : N * eo + n + 1],
                        )
                        sq = scratch.tile([128, D], F16, tag="sq")
                        nc.scalar.activation(
                            sq[:], xh, ACTF.Square, bias=zero_sb[:],
                            accum_out=ssqs2[:, N * eo + n : N * eo + n + 1],
                        )

                # softmax over n, both parities merged into [128, 2*N] ops
                # (cols = 8*eo + n); rms via sqrt + reciprocal
                sqs = smalls.tile([128, 2 * N], F32, tag="sqs")
                nc.scalar.activation(
                    sqs[:], ssqs2[:], ACTF.Sqrt, bias=eps_sb[:], scale=1.0 / D
                )
                rms = smalls.tile([128, 2 * N], F32, tag="rms")
                nc.vector.reciprocal(rms[:], sqs[:])
                logits = smalls.tile([128, 2 * N], F32, tag="logits")
                nc.vector.tensor_mul(logits[:], dots2[:], rms[:])
                l3 = logits[:].rearrange("p (e n) -> p e n", n=N)
                negmax = smalls.tile([128, 2], F32, tag="negmax")
                nc.vector.tensor_reduce(
                    negmax[:], l3, axis=mybir.AxisListType.X,
                    op=ALU.max, negate=True,
                )
                shifted = smalls.tile([128, 2 * N], F32, tag="shifted")
                s3 = shifted[:].rearrange("p (e n) -> p e n", n=N)
                nc.vector.tensor_tensor(
                    s3, l3, negmax[:].unsqueeze(2).broadcast_to([128, 2, N]),
                    ALU.add,
                )
                expd = smalls.tile([128, 2 * N], F32, tag="expd")
                nc.scalar.activation(expd[:], shifted[:], ACTF.Exp, bias=zero_sb[:])
                e3 = expd[:].rearrange("p (e n) -> p e n", n=N)
                sums = smalls.tile([128, 2], F32, tag="sums")
                nc.vector.tensor_reduce(
                    sums[:], e3, axis=mybir.AxisListType.X, op=ALU.add
                )
                rsums = smalls.tile([128, 2], F32, tag="rsums")
                nc.vector.reciprocal(rsums[:], sums[:])
                wts = smalls.tile([128, 2 * N], F32, tag="wts")
                w3 = wts[:].rearrange("p (e n) -> p e n", n=N)
                nc.vector.tensor_tensor(
                    w3, e3, rsums[:].unsqueeze(2).broadcast_to([128, 2, N]),
                    ALU.mult,
                )
                # wd[:, 128n+m] = wts[:, 8eo+n] * id[:, m] -> diag stationaries
                wds = []
                for eo in range(2):
                    wd = wdpool.tile([128, N * 128], F16, tag=f"wd{eo}")
                    for n in range(N):
                        nc.vector.tensor_scalar(
                            out=wd[:, 128 * n : 128 * (n + 1)],
                            in0=id_sb[:],
                            scalar1=wts[:, N * eo + n : N * eo + n + 1],
                            scalar2=None,
                            op0=ALU.mult,
                        )
                    wds.append(wd)

                # weighted sum: per (parity, D-half), 8 accumulating diag
                # matmuls per 512-chunk
                osb = outpool.tile([128, 2 * D], BF16, tag="osb")
                for eo in range(2):
                    for h in range(2):
                        psb = psum_big_pool.tile([128, D // 2], F32, tag="psb")
                        for n in range(N):
                            lhsT = wds[eo][:, 128 * n : 128 * (n + 1)]
                            for kk in range(D // NCHUNK // 2):
                                k = h * (D // NCHUNK // 2) + kk
                                nc.tensor.matmul(
                                    psb[:, NCHUNK * kk : NCHUNK * (kk + 1)],
                                    lhsT,
                                    xt[n][:, D * eo + NCHUNK * k
                                           : D * eo + NCHUNK * (k + 1)],
                                    start=(n == 0),
                                    stop=(n == N - 1),
                                )
                        # split the PSUM->SBUF copies across ACT and DVE
                        eng = nc.scalar.copy if h == 0 else nc.vector.tensor_copy
                        eng(
                            osb[:, D * eo + h * (D // 2)
                                   : D * eo + (h + 1) * (D // 2)],
                            psb[:],
                        )
                # partition p holds rows (2p, 2p+1): contiguous 1 MiB store
                dst = (
                    out_dram[BR * b : BR * (b + 1), :]
                    .rearrange("(p two) d -> p (two d)", two=2)
                )
                nc.scalar.dma_start(dst, osb[:])

            def emit_half_block(r0):
                """128-row block, one row per partition ([128, D] tiles).

                Used for the final rows so the end-of-kernel dependency
                chain (softmax -> matmuls -> store) is half as long.
                """
                xt = []
                dots = smalls.tile([128, N], F32, tag="dotsh")
                ssqs = smalls.tile([128, N], F32, tag="ssqsh")
                for n in range(N):
                    xfull = xpool.tile([128, 2 * D], F16, tag="x")
                    x = xfull[:, 0:D]
                    nc.gpsimd.dma_start(x, v_dram[n, r0 : r0 + 128, :])
                    xt.append(xfull)
                    prod = scratch.tile([128, D], F16, tag="prod")
                    nc.vector.scalar_tensor_tensor(
                        out=prod[:], in0=x, scalar=1.0, in1=qw_sb[:],
                        op0=ALU.mult, op1=ALU.mult,
                        accum_out=dots[:, n : n + 1],
                    )
                    sq = scratch.tile([128, D], F16, tag="sq")
                    nc.scalar.activation(
                        sq[:], x, ACTF.Square, bias=zero_sb[:],
                        accum_out=ssqs[:, n : n + 1],
                    )

                lns = smalls.tile([128, N], F32, tag="lnsh")
                nc.scalar.activation(
                    lns[:], ssqs[:], ACTF.Ln, bias=eps_sb[:], scale=1.0 / D
                )
                rms = smalls.tile([128, N], F32, tag="rmsh")
                nc.scalar.activation(
                    rms[:], lns[:], ACTF.Exp, bias=zero_sb[:], scale=-0.5
                )
                logits = smalls.tile([128, N], F32, tag="logitsh")
                nc.vector.tensor_mul(logits[:], dots[:], rms[:])
                negmax = smalls.tile([128, 1], F32, tag="negmaxh")
                nc.vector.tensor_reduce(
                    negmax[:], logits[:], axis=mybir.AxisListType.X,
                    op=ALU.max, negate=True,
                )
                shifted = smalls.tile([128, N], F32, tag="shiftedh")
                nc.vector.tensor_tensor(
                    shifted[:], logits[:], negmax[:].broadcast_to([128, N]),
                    ALU.add,
                )
                expd = smalls.tile([128, N], F32, tag="expdh")
                nc.scalar.activation(expd[:], shifted[:], ACTF.Exp, bias=zero_sb[:])
                sums = smalls.tile([128, 1], F32, tag="sumsh")
                nc.vector.tensor_reduce(
                    sums[:], expd[:], axis=mybir.AxisListType.X, op=ALU.add
                )
                rsums = smalls.tile([128, 1], F32, tag="rsumsh")
                nc.vector.reciprocal(rsums[:], sums[:])
                wts = smalls.tile([128, N], F32, tag="wtsh")
                nc.vector.tensor_tensor(
                    wts[:], expd[:], rsums[:].broadcast_to([128, N]), ALU.mult
                )
                wd = wdpool.tile([128, N * 128], F16, tag="wdh")
                for n in range(N):
                    nc.vector.tensor_scalar(
                        out=wd[:, 128 * n : 128 * (n + 1)],
                        in0=id_sb[:],
                        scalar1=wts[:, n : n + 1],
                        scalar2=None,
                        op0=ALU.mult,
                    )

                osb = outpool.tile([128, D], BF16, tag="osbh")
                for h in range(2):
                    psb = psum_big_pool.tile([128, D // 2], F32, tag="psb")
                    for n in range(N):
                        lhsT = wd[:, 128 * n : 128 * (n + 1)]
                        for kk in range(D // NCHUNK // 2):
                            k = h * (D // NCHUNK // 2) + kk
                            nc.tensor.matmul(
                                psb[:, NCHUNK * kk : NCHUNK * (kk + 1)],
                                lhsT,
                                xt[n][:, NCHUNK * k : NCHUNK * (k + 1)],
                                start=(n == 0),
                                stop=(n == N - 1),
                            )
                    eng = nc.scalar.copy if h == 0 else nc.vector.tensor_copy
                    eng(osb[:, h * (D // 2) : (h + 1) * (D // 2)], psb[:])
                nc.scalar.dma_start(out_dram[r0 : r0 + 128, :], osb[:])

            for b in range(nb - 1):
                emit_block(b)
            emit_half_block(BR * (nb - 1))
            emit_half_block(BR * (nb - 1) + 128)

    nc.compile()
    return nc


def prepare_in_maps(V, key_norm_weight, pseudo_query, rows_per_core=RPC,
                    n_cores=N_CORES):
    qw = (np.asarray(key_norm_weight, dtype=np.float32)
          * np.asarray(pseudo_query, dtype=np.float32)).astype(np.float16)
    qw_b = np.ascontiguousarray(np.broadcast_to(qw, (128, D)))
    ident = np.eye(128, dtype=np.float16)
    vf = np.ascontiguousarray(np.asarray(V, dtype=np.float32)).reshape(N, -1, D)
    in_maps = []
    for c in range(n_cores):
        sl = np.ascontiguousarray(
            vf[:, c * rows_per_core : (c + 1) * rows_per_core, :]
        )
        in_maps.append({"V": sl, "QW": qw_b, "ID": ident})
    return in_maps


_PROGRAM_CACHE = {}


def _get_program():
    key = (RPC,)
    if key not in _PROGRAM_CACHE:
        _PROGRAM_CACHE[key] = build_program(RPC, debug=False)
    return _PROGRAM_CACHE[key]


def run(V, key_norm_weight, pseudo_query, trace=False, **trace_kwargs):
    nc = _get_program()
    in_maps = prepare_in_maps(V, key_norm_weight, pseudo_query)
    res = run_bass_kernel_spmd(
        nc, in_maps, list(range(N_CORES)), trace=trace, **trace_kwargs
    )
    out = np.empty((R_TOTAL, D), dtype=np.float32)
    for c in range(N_CORES):
        out[c * RPC : (c + 1) * RPC, :] = np.asarray(
            res.results[c]["OUT"]
        ).astype(np.float32)
    return out.reshape(B, T, D), res


def kernel(V, key_norm_weight, pseudo_query):
    out, _ = run(V, key_norm_weight, pseudo_query, trace=False)
    return out
